# revision 57
# baseline (speedup 1.0000x reference)
"""Trainium2 Bass kernel for nn_DigitCap (capsule DigitCaps layer).

Math: the reference's routing loop is degenerate — softmax over a size-1
axis is exactly 1.0, so c_ij == 1 on every iteration and the output only
depends on s[b,l,o] = sum_{p,n} W[0,p,l,o,n] * x[b,n,p], followed by the
squash nonlinearity (norm taken over the L axis, faithful to the source):

    m2[b,o]    = sum_l s[b,l,o]^2
    out[b,l,o] = s[b,l,o] * sqrt(m2[b,o]) / (1 + m2[b,o])

This collapses to one (256 x 9216) @ (9216 x 160) matmul plus a tiny
elementwise epilogue.

Sharding over 8 NeuronCores — shipped mode "bp4": a 4-way batch x 2-way
output-capsule grid in bf16, with NO collective (on this stack every
8-rank collective costs 50-65us of ncfw control-plane latency regardless
of payload, measured AR/AG/RS/A2A).  Each core computes a (64 batch x 80
col) block: bf16 inputs halve the bytes (rel err ~3e-3, vs the 2e-2
gate) and the 4x2 grid replicates x only 2x and W only 4x, so per-core
traffic is 2.65 MB vs 7.1 MB for the f32 8-way-batch layout.  Splitting
the 160 output columns along O (f = o*10 + l, all 10 l's per core) keeps
the squash l-reduction core-local and an innermost-axis DVE reduce.

Per-core kernel: W and x are host-interleaved into ONE packed bf16
stream, per k-chunk the block [w(80 cols) | xt(64 cols)], so DMA
delivery order == PE consumption order and every transfer is a
contiguous per-partition run.  The stream moves in 12 six-chunk
dma_starts (1728B runs) alternating between the two HWDGE paths (sync/
scalar); measured ring behaviour: ~26 GB/s per ring x 16 rings,
~320-340 GB/s/core aggregate (the cap), with each ring serving each
source FIFO — delivery stays in consumption order at fine granularity
so the in-order PE consumer never stalls long.  All 72 k-chunk matmuls
[128,64,80] accumulate into ONE [64,80] PSUM tile: the 64-col
LDWEIGHTS (~60ns) hides behind the previous matmul's 80-col stream
(~67ns) in the PE weight double-buffer even at a single tile position,
sustaining one matmul per ~67ns at the throttled 1.2 GHz PE clock (the
HAM never un-throttles mid-kernel; warm-up matmuls measured as pure
loss and were removed).  No column tiling means no psum strip-sum — no
selection matmul, no bf16 CAST — so the serial epilogue is just a
PSUM->SBUF copy plus squash, with Sqrt as the ONLY ACT function (ACT's
table RAM holds one table; any second function reloads 1.28us
mid-epilogue).

Measured on HW: ~23.7-24.6us end-to-end (run-to-run spread ~1.5-2.5us,
machine-state drift included; interleaved A/B against the column-tiled
strip-sum variant "bp3" shows bp4 ~1us faster), vs 38.0us for the f32
bp2 baseline.  Fixed framework cost (semaphore init/drain storms,
TENSOR_LOAD, ring flush) measures ~14us for a minimal kernel, so the
compute+DMA body is within ~2us of the HBM-rate floor for this
sharding.  Alternate modes kept for reference: "bp3" (column-tiled +
strip-sum, ~24.5-25.5us), "bp2" (f32 8-way batch, 38us), "bp"
(unpacked, 52us), "a2a"/"rs"/"ar"/"ag" (K-sharded + collectives,
87-105us).

The host converts the gathered (256,160) result back to (256, 10, 16).
"""

import numpy as np

B, N, P, L, O = 256, 8, 1152, 10, 16
NCORES = 8
KC = P // 128          # 9 k-chunks of 128 per core
BB = B // NCORES       # 32 batch rows per core in the scatter modes
LO = L * O             # 160

MODE = "bp4"

GP = 4                 # col-tiled k-chunks per PE pass in "bp" mode
NPASS = N * P // 128 // GP   # 18 passes over the full K for one core

# bp3: 4-way batch x 2-way output-capsule sharding, bf16 inputs.
B4 = B // 4            # 64 batch rows per core
O2 = O // 2            # 8 output capsules per core
FO = O2 * L            # 80 output columns per core (f = o_local*10 + l)
KC3 = N * P // 128     # 72 k-chunks of 128
GP3 = 2                # col-tiled k-chunks per PE pass (two 64-col groups)
NP3 = KC3 // GP3       # 36 passes
WXC = GP3 * FO + GP3 * B4   # 288 packed cols per pass: [w | xt]

# bp4: like bp3 but ONE 64-col PE group (no column tiling) — the
# 64-col LDWEIGHTS (~60ns) still hides behind the previous matmul's
# 80-col stream (~67ns) via the PE's weight double-buffer, the psum
# strip-sum disappears, and with it the selection matmul + CAST.
WXC4 = FO + B4         # 144 packed cols per k-chunk: [w | xt]

_cache = {}


def _emit_squash(nc, mybir, post, s, nrows, idx, no=O):
    """Emit squash for an SBUF tile s of shape [nrows, no*L]; returns v tile."""
    f32 = mybir.dt.float32
    nf = no * L
    sq = post.tile([nrows, nf], f32, name=f"sq{idx}")
    m2 = post.tile([nrows, no], f32, name=f"m2{idx}")
    rt = post.tile([nrows, no], f32, name=f"rt{idx}")
    dn = post.tile([nrows, no], f32, name=f"dn{idx}")
    tf = post.tile([nrows, no], f32, name=f"tf{idx}")
    vv = post.tile([nrows, nf], f32, name=f"vv{idx}")
    nc.vector.tensor_mul(sq[:], s[:], s[:])
    nc.vector.reduce_sum(
        m2[:], sq[:].rearrange("b (o l) -> b o l", l=L),
        axis=mybir.AxisListType.X)
    nc.scalar.activation(rt[:], m2[:], mybir.ActivationFunctionType.Sqrt)
    nc.vector.tensor_scalar_add(dn[:], m2[:], 1.0)
    nc.vector.reciprocal(dn[:], dn[:])
    nc.vector.tensor_mul(tf[:], rt[:], dn[:])
    nc.vector.tensor_mul(
        vv[:].rearrange("b (o l) -> b o l", l=L),
        s[:].rearrange("b (o l) -> b o l", l=L),
        tf[:][:, :, None].broadcast_to([nrows, no, L]))
    return vv


def _build(mode=MODE):
    if mode in _cache:
        return _cache[mode]

    import concourse.bacc as bacc
    import concourse.mybir as mybir
    import concourse.tile as tile

    f32 = mybir.dt.float32
    nc = bacc.Bacc("TRN2", target_bir_lowering=False, debug=False,
                   num_devices=NCORES)
    if mode == "bp":
        return _build_bp(nc, mybir)
    if mode == "bp2":
        return _build_bp2(nc, mybir)
    if mode == "bp3":
        return _build_bp3(nc, mybir)
    if mode == "bp4":
        return _build_bp4(nc, mybir)
    if mode == "bp5":
        return _build_bp5(nc, mybir)
    xt_d = nc.dram_tensor("xt", [P, B], f32, kind="ExternalInput").ap()
    w_d = nc.dram_tensor("w", [P, LO], f32, kind="ExternalInput").ap()
    out_rows = BB if mode in ("rs", "a2a") else B
    out_d = nc.dram_tensor("out", [out_rows, LO], f32,
                           kind="ExternalOutput").ap()

    with tile.TileContext(nc) as tc:
        with (
            tc.tile_pool(name="io", bufs=3) as io_pool,
            tc.tile_pool(name="ps", bufs=1, space="PSUM") as ps_pool,
            tc.tile_pool(name="dram", bufs=1, space="DRAM") as dram_pool,
            tc.tile_pool(name="post", bufs=1) as post,
        ):
            xt_v = xt_d.rearrange("(c p) b -> c p b", p=128)
            w_v = w_d.rearrange("(c p) f -> c p f", p=128)
            ps0 = ps_pool.tile([128, LO], f32, name="ps0")
            ps1 = ps_pool.tile([128, LO], f32, name="ps1")
            for c in range(KC):
                xt_t = io_pool.tile([128, B], f32, tag="xt", name=f"xt{c}")
                w_t = io_pool.tile([128, LO], f32, tag="w", name=f"w{c}")
                nc.sync.dma_start(xt_t[:], xt_v[c])
                nc.sync.dma_start(w_t[:], w_v[c])
                nc.tensor.matmul(ps0[:], xt_t[:, 0:128], w_t[:],
                                 start=(c == 0), stop=(c == KC - 1))
                nc.tensor.matmul(ps1[:], xt_t[:, 128:256], w_t[:],
                                 start=(c == 0), stop=(c == KC - 1))

            partial = dram_pool.tile([B, LO], f32, name="partial")
            s0 = post.tile([128, LO], f32, name="s0")
            s1 = post.tile([128, LO], f32, name="s1")
            nc.vector.tensor_copy(s0[:], ps0[:])
            nc.vector.tensor_copy(s1[:], ps1[:])
            nc.sync.dma_start(partial[0:128, :], s0[:])
            nc.sync.dma_start(partial[128:256, :], s1[:])

            rg = [list(range(NCORES))]
            if mode == "ar":
                red = dram_pool.tile([B, LO], f32, name="red",
                                     addr_space="Shared")
                nc.gpsimd.collective_compute(
                    "AllReduce", mybir.AluOpType.add, replica_groups=rg,
                    ins=[partial.opt()], outs=[red.opt()])
                for h in range(2):
                    sh = post.tile([128, LO], f32, name=f"sh{h}")
                    nc.sync.dma_start(sh[:], red[128 * h:128 * (h + 1), :])
                    vv = _emit_squash(nc, mybir, post, sh, 128, h)
                    nc.sync.dma_start(out_d[128 * h:128 * (h + 1), :], vv[:])
            elif mode == "ag":
                red = dram_pool.tile([NCORES * B, LO], f32, name="red",
                                     addr_space="Shared")
                nc.gpsimd.collective_compute(
                    "AllGather", mybir.AluOpType.bypass, replica_groups=rg,
                    ins=[partial.opt()], outs=[red.opt()])
                red_v = red.rearrange("(r b) f -> b r f", b=B)
                for h in range(2):
                    r8 = post.tile([128, NCORES, LO], f32, name=f"r8{h}")
                    nc.sync.dma_start(r8[:], red_v[128 * h:128 * (h + 1)])
                    sh = post.tile([128, LO], f32, name=f"sh{h}")
                    nc.vector.reduce_sum(
                        sh[:], r8[:].rearrange("b r f -> b f r"),
                        axis=mybir.AxisListType.X)
                    vv = _emit_squash(nc, mybir, post, sh, 128, h)
                    nc.sync.dma_start(out_d[128 * h:128 * (h + 1), :], vv[:])
            elif mode == "rs":
                red = dram_pool.tile([BB, LO], f32, name="red")
                nc.gpsimd.collective_compute(
                    "ReduceScatter", mybir.AluOpType.add, replica_groups=rg,
                    ins=[partial.opt()], outs=[red.opt()])
                s = post.tile([BB, LO], f32, name="s")
                nc.sync.dma_start(s[:], red[:])
                vv = _emit_squash(nc, mybir, post, s, BB, 0)
                nc.sync.dma_start(out_d[:], vv[:])
            else:  # a2a
                red = dram_pool.tile([B, LO], f32, name="red")
                nc.gpsimd.collective_compute(
                    "AllToAll", mybir.AluOpType.bypass, replica_groups=rg,
                    ins=[partial.opt()], outs=[red.opt()])
                r8 = post.tile([BB, NCORES, LO], f32, name="r8")
                nc.sync.dma_start(r8[:], red.rearrange("(r b) f -> b r f",
                                                       b=BB))
                s = post.tile([BB, LO], f32, name="s")
                nc.vector.reduce_sum(
                    s[:], r8[:].rearrange("b r f -> b f r"),
                    axis=mybir.AxisListType.X)
                vv = _emit_squash(nc, mybir, post, s, BB, 0)
                nc.sync.dma_start(out_d[:], vv[:])

    nc.compile()
    _cache[mode] = nc
    return nc


def _build_bp(nc, mybir):
    """Batch-parallel: W replicated, batch sharded 8 x 32, no collective.

    PE efficiency at M=32 is recovered with 4x column tiling: each PE pass
    runs 4 k-chunks concurrently in the four 32-column groups of the array,
    accumulating into four disjoint 32-partition strips of one PSUM tile.
    The four strips are partial K-sums, added together on DVE at the end.
    DMA is split across both HWDGE queues (sync + scalar)."""
    import concourse.tile as tile

    f32 = mybir.dt.float32
    K = N * P
    xt_d = nc.dram_tensor("xt", [K, BB], f32, kind="ExternalInput").ap()
    w_d = nc.dram_tensor("w", [K, LO], f32, kind="ExternalInput").ap()
    sel_d = nc.dram_tensor("sel", [128, BB], f32, kind="ExternalInput").ap()
    out_d = nc.dram_tensor("out", [BB, LO], f32, kind="ExternalOutput").ap()

    with tile.TileContext(nc) as tc:
        with (
            tc.tile_pool(name="io", bufs=3) as io_pool,
            tc.tile_pool(name="ps", bufs=1, space="PSUM") as ps_pool,
            tc.tile_pool(name="post", bufs=1) as post,
        ):
            xt_v = xt_d.rearrange("(g j p) m -> g p j m", j=GP, p=128)
            w_v = w_d.rearrange("(g j p) f -> g p j f", j=GP, p=128)
            sel_t = post.tile([128, BB], f32, name="sel_t")
            nc.scalar.dma_start(sel_t[:], sel_d[:])
            ps = ps_pool.tile([128, LO], f32, name="ps")
            for g in range(NPASS):
                xt_t = io_pool.tile([128, GP, BB], f32, tag="xt",
                                    name=f"xt{g}")
                w_t = io_pool.tile([128, GP, LO], f32, tag="w", name=f"w{g}")
                dma_eng = nc.sync if g % 2 == 0 else nc.scalar
                xt_eng = nc.scalar if g % 2 == 0 else nc.sync
                xt_eng.dma_start(xt_t[:], xt_v[g])
                dma_eng.dma_start(w_t[:], w_v[g])
                for j in range(GP):
                    nc.tensor.matmul(
                        ps[32 * j:32 * (j + 1), :], xt_t[:, j, :],
                        w_t[:, j, :], start=(g == 0), stop=(g == NPASS - 1),
                        tile_position=(0, 32 * j))

            # sum the four 32-partition strips: s = sel.T @ sp on the PE
            # (DVE cannot add across base partitions; walrus rejects it).
            sp = post.tile([128, LO], f32, name="sp")
            nc.vector.tensor_copy(sp[:], ps[:])
            ps2 = ps_pool.tile([BB, LO], f32, name="ps2")
            nc.tensor.matmul(ps2[:], sel_t[:], sp[:], start=True, stop=True)
            s = post.tile([BB, LO], f32, name="s")
            nc.vector.tensor_copy(s[:], ps2[:])
            vv = _emit_squash(nc, mybir, post, s, BB, 0)
            nc.sync.dma_start(out_d[:], vv[:])

    nc.compile()
    _cache["bp"] = nc
    return nc


def _build_bp2(nc, mybir):
    """Like bp, but inputs are host-packed so each PE pass's W/xt tile is a
    contiguous DRAM block (per-partition runs of 1280B/512B instead of
    640B/128B), and every W pass-load is split across both HWDGE queues."""
    import concourse.tile as tile

    f32 = mybir.dt.float32
    xt_d = nc.dram_tensor("xt", [128, NPASS * GP * BB], f32,
                          kind="ExternalInput").ap()
    w_d = nc.dram_tensor("w", [NPASS * 128, GP * LO], f32,
                         kind="ExternalInput").ap()
    sel_d = nc.dram_tensor("sel", [128, BB], f32, kind="ExternalInput").ap()
    out_d = nc.dram_tensor("out", [BB, LO], f32, kind="ExternalOutput").ap()

    with tile.TileContext(nc) as tc:
        with (
            tc.tile_pool(name="io", bufs=5) as io_pool,
            tc.tile_pool(name="ps", bufs=1, space="PSUM") as ps_pool,
            tc.tile_pool(name="post", bufs=1) as post,
        ):
            # DMA granularity: PR passes per issue (fewer, larger transfers —
            # each dma_start costs ~670ns of issue time on its HWDGE engine,
            # and the kernel-teardown sem storm scales with instruction count).
            # The first group is a single pass so the PE can start sooner.
            PR = 3
            groups = [1] + [PR] * ((NPASS - 1) // PR) + \
                     ([NPASS - 1 - (NPASS - 1) // PR * PR] or [])
            groups = [n for n in groups if n]
            w_vp = w_d.rearrange("(g p) f -> g p f", p=128)
            sel_t = post.tile([128, BB], f32, name="sel_t")
            nc.scalar.dma_start(sel_t[:], sel_d[:])
            # x is tiny (9.2KB/partition): keep it SBUF-resident, loaded by
            # two early DMAs instead of one per group — fewer issues and no
            # xt dependency in the W streaming pipeline.
            XA = 7 * GP * BB
            xt_all = post.tile([128, NPASS * GP * BB], f32, name="xt_all")
            nc.scalar.dma_start(xt_all[:, 0:XA], xt_d[:, 0:XA])
            ps = ps_pool.tile([128, LO], f32, name="ps")
            # PE warm-up: ~4us of dummy matmuls on the tiny sel tile while
            # the first W loads are in flight, so the HAM un-throttles the
            # PE clock (1.2 -> 2.4 GHz) before the real passes start.
            warm = ps_pool.tile([BB, BB], f32, name="warm")
            for _ in range(10):
                nc.tensor.matmul(warm[:], sel_t[:, 0:BB], sel_t[:, 0:BB],
                                 start=True, stop=True)
            g0 = 0
            for gi, npg in enumerate(groups):
                w_t = io_pool.tile([128, npg, GP * LO], f32, tag="w",
                                   name=f"w{gi}")
                ws = w_vp[g0:g0 + npg].rearrange("h p f -> p h f")
                e0, e1 = (nc.sync, nc.scalar) if gi % 2 == 0 else \
                         (nc.scalar, nc.sync)
                if npg == 1:
                    half = GP * LO // 2
                    e0.dma_start(w_t[:, 0, 0:half], ws[:, 0, 0:half])
                    e1.dma_start(w_t[:, 0, half:], ws[:, 0, half:])
                else:
                    # first-needed pass on e0, rest on e1
                    e0.dma_start(w_t[:, 0:1, :], ws[:, 0:1, :])
                    e1.dma_start(w_t[:, 1:npg, :], ws[:, 1:npg, :])
                if gi == 0:
                    nc.sync.dma_start(xt_all[:, XA:], xt_d[:, XA:])
                for h in range(npg):
                    g = g0 + h
                    for j in range(GP):
                        c = g * GP + j
                        nc.tensor.matmul(
                            ps[32 * j:32 * (j + 1), :],
                            xt_all[:, BB * c:BB * (c + 1)],
                            w_t[:, h, LO * j:LO * (j + 1)],
                            start=(g == 0), stop=(g == NPASS - 1),
                            tile_position=(0, 32 * j))
                g0 += npg

            sp = post.tile([128, LO], f32, name="sp")
            nc.vector.tensor_copy(sp[:], ps[:])
            ps2 = ps_pool.tile([BB, LO], f32, name="ps2")
            nc.tensor.matmul(ps2[:], sel_t[:], sp[:], start=True, stop=True)
            s = post.tile([BB, LO], f32, name="s")
            nc.vector.tensor_copy(s[:], ps2[:])
            vv = _emit_squash(nc, mybir, post, s, BB, 0)
            nc.sync.dma_start(out_d[:], vv[:])

    nc.compile()
    _cache["bp2"] = nc
    return nc


def _build_bp3(nc, mybir):
    """4-way batch x 2-way output-capsule sharding, bf16 inputs.

    Each core computes s[b, f] for 64 batch rows and 80 output columns
    (8 of the 16 o-capsules, all 10 l's; the squash l-reduction stays
    core-local).  Per-core traffic drops from 7.1 MB (bp2) to 2.65 MB:
    bf16 halves the bytes and the 4x2 grid replicates x only 2x and W
    only 4x instead of 8x.

    W and x are host-interleaved into ONE packed stream wx: per PE pass
    g the block [w(2 chunks, 160 cols) | xt(2 chunks, 128 cols)], so DMA
    delivery order == PE consumption order and every transfer is one
    contiguous per-partition run, moved as 12 3-pass dma_starts
    alternating between the two HWDGE paths.  M=64 PE efficiency is
    recovered with 2x column tiling (tile_position=(0,64j)); the two
    64-partition strips are summed by a small selection-matrix matmul
    as in bp2.
    """
    import concourse.tile as tile

    f32 = mybir.dt.float32
    bf16 = mybir.dt.bfloat16
    wx_d = nc.dram_tensor("wx", [128, NP3 * WXC], bf16,
                          kind="ExternalInput").ap()
    sel_d = nc.dram_tensor("sel", [128, B4], bf16, kind="ExternalInput").ap()
    out_d = nc.dram_tensor("out", [B4, FO], f32, kind="ExternalOutput").ap()

    with tile.TileContext(nc) as tc:
        with (
            tc.tile_pool(name="io", bufs=12) as io_pool,
            tc.tile_pool(name="ps", bufs=1, space="PSUM") as ps_pool,
            tc.tile_pool(name="post", bufs=1) as post,
        ):
            # Sequential small groups: one dma_start per group (128
            # descriptors, npg*576B contiguous per-partition runs),
            # alternating between the two HWDGE paths.  The 16 hardware
            # rings serve each source FIFO but arbitrate BETWEEN the
            # two sources in bursts, so LARGE alternated groups arrive
            # out of order and stall the in-order PE consumer; at
            # 3-pass granularity the skew stays below the PE's slack
            # while the alternation fills each source's ring re-arm
            # gaps (measured best of 2/3/4/6-pass x single/dual-source
            # variants, ~320 GB/s/core aggregate — the cap).  No
            # warm-up: the HAM never un-throttles the PE clock
            # mid-kernel (bp2's ramp fired at t=33us, after its
            # matmuls), so warm-up matmuls only delayed the first pass.
            groups = [3] * 12
            assert sum(groups) == NP3
            wx_v = wx_d.rearrange("p (g c) -> p g c", c=WXC)
            sel_t = post.tile([128, B4], bf16, name="sel_t")
            ps = ps_pool.tile([128, FO], f32, name="ps")
            g0 = 0
            for gi, npg in enumerate(groups):
                wx_t = io_pool.tile([128, npg, WXC], bf16, tag="wx",
                                    name=f"wx{gi}")
                e = nc.sync if gi % 2 == 0 else nc.scalar
                e.dma_start(wx_t[:], wx_v[:, g0:g0 + npg])
                if gi == 0:
                    # sel is only needed for the final strip-sum; issue it
                    # on the other HWDGE path so pass-0 data leads.
                    nc.scalar.dma_start(sel_t[:], sel_d[:])
                for h in range(npg):
                    g = g0 + h
                    for j in range(GP3):
                        nc.tensor.matmul(
                            ps[B4 * j:B4 * (j + 1), :],
                            wx_t[:, h, GP3 * FO + B4 * j:
                                 GP3 * FO + B4 * (j + 1)],
                            wx_t[:, h, FO * j:FO * (j + 1)],
                            start=(g == 0), stop=(g == NP3 - 1),
                            tile_position=(0, B4 * j))
                g0 += npg

            # sum the two 64-partition strips: s = sel.T @ sp on the PE
            # (bf16 so the strip-sum stream runs at full rate)
            sp = post.tile([128, FO], bf16, name="sp")
            nc.vector.tensor_copy(sp[:], ps[:])
            ps2 = ps_pool.tile([B4, FO], f32, name="ps2")
            nc.tensor.matmul(ps2[:], sel_t[:], sp[:], start=True, stop=True)
            # ACT's table RAM holds one table: keep Sqrt the only ACT
            # function so its table loads once early, never mid-epilogue.
            s = post.tile([B4, FO], f32, name="s")
            nc.vector.tensor_copy(s[:], ps2[:])
            vv = _emit_squash(nc, mybir, post, s, B4, 0, no=O2)
            # scalar's issue queue is long done by now; sync still owns
            # the end-barrier bookkeeping, so the out store leaves sooner
            # from scalar.
            nc.scalar.dma_start(out_d[:], vv[:])

    nc.compile()
    _cache["bp3"] = nc
    return nc


def _build_bp4(nc, mybir):
    """bp3's sharding/stream with a single 64-col PE group.

    All 72 k-chunk matmuls accumulate into one [64, 80] PSUM tile at
    tile_position (0,0); the 64-col LDWEIGHTS hides behind the previous
    matmul's 80-col stream in the PE weight double-buffer, so the
    cadence matches bp3's column-tiled form while the strip-sum
    (selection matmul + bf16 CAST + sel DMA) disappears from the serial
    epilogue.
    """
    import concourse.tile as tile

    f32 = mybir.dt.float32
    bf16 = mybir.dt.bfloat16
    wx_d = nc.dram_tensor("wx", [128, KC3 * WXC4], bf16,
                          kind="ExternalInput").ap()
    out_d = nc.dram_tensor("out", [B4, FO], f32, kind="ExternalOutput").ap()

    with tile.TileContext(nc) as tc:
        with (
            tc.tile_pool(name="io", bufs=12) as io_pool,
            tc.tile_pool(name="ps", bufs=1, space="PSUM") as ps_pool,
            tc.tile_pool(name="post", bufs=1) as post,
        ):
            # same delivery scheme as bp3: 12 groups (6 chunks each,
            # 1728B contiguous per-partition runs) alternating between
            # the two HWDGE paths.
            groups = [6] * 12
            assert sum(groups) == KC3
            wx_v = wx_d.rearrange("p (g c) -> p g c", c=WXC4)
            ps = ps_pool.tile([B4, FO], f32, name="ps")
            g0 = 0
            for gi, npg in enumerate(groups):
                wx_t = io_pool.tile([128, npg, WXC4], bf16, tag="wx",
                                    name=f"wx{gi}")
                e = nc.sync if gi % 2 == 0 else nc.scalar
                e.dma_start(wx_t[:], wx_v[:, g0:g0 + npg])
                for h in range(npg):
                    c = g0 + h
                    nc.tensor.matmul(
                        ps[:], wx_t[:, h, FO:WXC4], wx_t[:, h, 0:FO],
                        start=(c == 0), stop=(c == KC3 - 1))
                g0 += npg

            s = post.tile([B4, FO], f32, name="s")
            nc.vector.tensor_copy(s[:], ps[:])
            vv = _emit_squash(nc, mybir, post, s, B4, 0, no=O2)
            # single store on scalar: keeping sync out of the output
            # path lets its teardown drain run early, concurrent with
            # the store (a 2-engine split store measured slower).
            nc.scalar.dma_start(out_d[:], vv[:])

    nc.compile()
    _cache["bp4"] = nc
    return nc


def _build_bp5(nc, mybir):
    """bp4 + head-start: group 0's DMA is emitted before TileContext so
    it issues from the 'main' block (~1us earlier ring start); the first
    matmuls wait on a manual completion semaphore."""
    import concourse.tile as tile

    f32 = mybir.dt.float32
    bf16 = mybir.dt.bfloat16
    wx_d = nc.dram_tensor("wx", [128, KC3 * WXC4], bf16,
                          kind="ExternalInput").ap()
    out_d = nc.dram_tensor("out", [B4, FO], f32, kind="ExternalOutput").ap()
    wx_v = wx_d.rearrange("p (g c) -> p g c", c=WXC4)

    GH = 6
    wx0 = nc.alloc_sbuf_tensor("wx0", [128, GH * WXC4], bf16)
    esem = nc.alloc_semaphore("early_wx0")
    wx0_v = wx0.ap().rearrange("p (h c) -> p h c", c=WXC4)
    nc.sync.dma_start(wx0_v[:], wx_v[:, 0:GH])

    with tile.TileContext(nc) as tc:
        with (
            tc.tile_pool(name="io", bufs=12) as io_pool,
            tc.tile_pool(name="ps", bufs=1, space="PSUM") as ps_pool,
            tc.tile_pool(name="post", bufs=1) as post,
        ):
            groups = [GH] * 11
            assert GH + sum(groups) == KC3
            ps = ps_pool.tile([B4, FO], f32, name="ps")
            # sync quiesces its queues (covers the main-block DMA) and
            # signals; the DMA's own then_inc is not modeled for
            # out-of-tile transfers.
            nc.sync.drain()
            nc.sync.sem_inc(esem, 1)
            nc.tensor.wait_ge(esem, 1)
            for h in range(GH):
                nc.tensor.matmul(
                    ps[:], wx0_v[:, h, FO:WXC4], wx0_v[:, h, 0:FO],
                    start=(h == 0), stop=False)
            g0 = GH
            for gi, npg in enumerate(groups):
                wx_t = io_pool.tile([128, npg, WXC4], bf16, tag="wx",
                                    name=f"wx{gi}")
                e = nc.scalar if gi % 2 == 0 else nc.sync
                e.dma_start(wx_t[:], wx_v[:, g0:g0 + npg])
                for h in range(npg):
                    c = g0 + h
                    nc.tensor.matmul(
                        ps[:], wx_t[:, h, FO:WXC4], wx_t[:, h, 0:FO],
                        start=False, stop=(c == KC3 - 1))
                g0 += npg

            s = post.tile([B4, FO], f32, name="s")
            nc.vector.tensor_copy(s[:], ps[:])
            vv = _emit_squash(nc, mybir, post, s, B4, 0, no=O2)
            nc.scalar.dma_start(out_d[:], vv[:])

    nc.compile()
    _cache["bp5"] = nc
    return nc


def _prep_inputs(x, W, mode=MODE):
    x = np.asarray(x, dtype=np.float32)
    W = np.asarray(W, dtype=np.float32)
    if mode in ("bp4", "bp5"):
        import ml_dtypes
        bf16 = ml_dtypes.bfloat16
        wf = np.ascontiguousarray(
            W[0].transpose(3, 0, 2, 1).reshape(N * P, LO))
        wpass = {}
        for ci in range(2):
            wpass[ci] = wf[:, FO * ci:FO * (ci + 1)].reshape(KC3, 128, FO)
        xpass = {}
        for ri in range(4):
            xt = x[B4 * ri:B4 * (ri + 1)].reshape(B4, N * P).T  # (9216, 64)
            xpass[ri] = xt.reshape(KC3, 128, B4)
        in_maps = []
        for i in range(NCORES):
            ri, ci = i // 2, i % 2
            wx = np.concatenate([wpass[ci], xpass[ri]], axis=2)
            wx = np.ascontiguousarray(
                wx.transpose(1, 0, 2).reshape(128, KC3 * WXC4)).astype(bf16)
            in_maps.append({"wx": wx})
        return in_maps
    if mode == "bp3":
        import ml_dtypes
        bf16 = ml_dtypes.bfloat16
        # wf rows k=(n,p), cols f=o*10+l
        wf = np.ascontiguousarray(
            W[0].transpose(3, 0, 2, 1).reshape(N * P, LO))
        sel = np.zeros((128, B4), np.float32)
        sel[np.arange(128), np.arange(128) % B4] = 1.0
        sel = sel.astype(bf16)
        # per-pass packed blocks, shared pieces computed once
        wpass = {}
        for ci in range(2):
            wc = wf[:, FO * ci:FO * (ci + 1)].reshape(NP3, GP3 * 128, FO)
            wpass[ci] = wc.reshape(NP3, GP3, 128, FO).transpose(
                0, 2, 1, 3).reshape(NP3, 128, GP3 * FO)
        xpass = {}
        for ri in range(4):
            xt = x[B4 * ri:B4 * (ri + 1)].reshape(B4, N * P).T  # (9216, 64)
            xpass[ri] = xt.reshape(NP3, GP3, 128, B4).transpose(
                0, 2, 1, 3).reshape(NP3, 128, GP3 * B4)
        in_maps = []
        for i in range(NCORES):
            ri, ci = i // 2, i % 2
            wx = np.concatenate([wpass[ci], xpass[ri]], axis=2)
            wx = np.ascontiguousarray(
                wx.transpose(1, 0, 2).reshape(128, NP3 * WXC)).astype(bf16)
            in_maps.append({"wx": wx, "sel": sel})
        return in_maps
    if mode == "bp2":
        # pack so each pass's tile is one contiguous DRAM block:
        # packed[g, p, j*D+d] = flat[128*(GP*g+j)+p, d]
        wf = np.ascontiguousarray(
            W[0].transpose(3, 0, 2, 1).reshape(N * P, LO))
        w2 = np.ascontiguousarray(
            wf.reshape(NPASS, GP, 128, LO).transpose(0, 2, 1, 3)
            .reshape(NPASS * 128, GP * LO))
        sel = np.zeros((128, BB), np.float32)
        sel[np.arange(128), np.arange(128) % BB] = 1.0
        in_maps = []
        for i in range(NCORES):
            xt = x[BB * i:BB * (i + 1)].reshape(BB, N * P).T  # (9216, 32)
            x2 = np.ascontiguousarray(
                xt.reshape(NPASS * GP, 128, BB).transpose(1, 0, 2)
                .reshape(128, NPASS * GP * BB))
            in_maps.append({"xt": x2, "w": w2, "sel": sel})
        return in_maps
    if mode == "bp":
        # xt = per-core batch-slice of x, flattened (b, n*p) and transposed;
        # w = full W with rows k=(n,p), cols f=o*10+l — identical per core.
        wf = np.ascontiguousarray(
            W[0].transpose(3, 0, 2, 1).reshape(N * P, LO))    # (9216, 160)
        sel = np.zeros((128, BB), np.float32)
        sel[np.arange(128), np.arange(128) % BB] = 1.0
        in_maps = []
        for i in range(NCORES):
            xs = x[BB * i:BB * (i + 1)].reshape(BB, N * P)
            in_maps.append({"xt": np.ascontiguousarray(xs.T), "w": wf,
                            "sel": sel})
        return in_maps
    in_maps = []
    for i in range(NCORES):
        xt = np.ascontiguousarray(x[:, i, :].T)               # (1152, 256)
        w = np.ascontiguousarray(
            W[0, :, :, :, i].transpose(0, 2, 1).reshape(P, LO))  # (1152, 160)
        in_maps.append({"xt": xt, "w": w})
    return in_maps


def _postprocess(results, mode=MODE):
    if mode in ("bp3", "bp4", "bp5"):
        full = np.zeros((B, LO), np.float32)
        for i in range(NCORES):
            ri, ci = i // 2, i % 2
            full[B4 * ri:B4 * (ri + 1), FO * ci:FO * (ci + 1)] = \
                results[i]["out"]
        return np.ascontiguousarray(
            full.reshape(B, O, L).transpose(0, 2, 1))
    if mode in ("rs", "a2a", "bp", "bp2"):
        full = np.concatenate([results[i]["out"] for i in range(NCORES)],
                              axis=0)
    else:
        full = results[0]["out"]
    return np.ascontiguousarray(
        full.reshape(B, O, L).transpose(0, 2, 1))             # (256, 10, 16)


def kernel(x, W):
    from concourse.bass_utils import run_bass_kernel_spmd

    nc = _build(MODE)
    res = run_bass_kernel_spmd(nc, _prep_inputs(x, W, MODE),
                               core_ids=list(range(NCORES)))
    return _postprocess(res.results)



# revision 58
# speedup vs baseline: 1.0960x; 1.0960x over previous
"""Trainium2 Bass kernel for nn_DigitCap (capsule DigitCaps layer).

Math: the reference's routing loop is degenerate — softmax over a size-1
axis is exactly 1.0, so c_ij == 1 on every iteration and the output only
depends on s[b,l,o] = sum_{p,n} W[0,p,l,o,n] * x[b,n,p], followed by the
squash nonlinearity (norm taken over the L axis, faithful to the source):

    m2[b,o]    = sum_l s[b,l,o]^2
    out[b,l,o] = s[b,l,o] * sqrt(m2[b,o]) / (1 + m2[b,o])

This collapses to one (256 x 9216) @ (9216 x 160) matmul plus a tiny
elementwise epilogue.

Sharding over 8 NeuronCores — shipped mode "bp4": a 4-way batch x 2-way
output-capsule grid in bf16, with NO collective (on this stack every
8-rank collective costs 50-65us of ncfw control-plane latency regardless
of payload, measured AR/AG/RS/A2A).  Each core computes a (64 batch x 80
col) block: bf16 inputs halve the bytes (rel err ~3e-3, vs the 2e-2
gate) and the 4x2 grid replicates x only 2x and W only 4x, so per-core
traffic is 2.65 MB vs 7.1 MB for the f32 8-way-batch layout.  Splitting
the 160 output columns along O (f = o*10 + l, all 10 l's per core) keeps
the squash l-reduction core-local and an innermost-axis DVE reduce.

Per-core kernel: W and x are host-interleaved into ONE packed bf16
stream, per k-chunk the block [w(80 cols) | xt(64 cols)], so DMA
delivery order == PE consumption order and every transfer is a
contiguous per-partition run.  The stream moves in 12 six-chunk
dma_starts (1728B runs) alternating between the two HWDGE paths (sync/
scalar); measured ring behaviour: ~26 GB/s per ring x 16 rings,
~320-340 GB/s/core aggregate (the cap), with each ring serving each
source FIFO — delivery stays in consumption order at fine granularity
so the in-order PE consumer never stalls long.  All 72 k-chunk matmuls
[128,64,80] accumulate into ONE [64,80] PSUM tile: the 64-col
LDWEIGHTS (~60ns) hides behind the previous matmul's 80-col stream
(~67ns) in the PE weight double-buffer even at a single tile position,
sustaining one matmul per ~67ns at the throttled 1.2 GHz PE clock (the
HAM never un-throttles mid-kernel; warm-up matmuls measured as pure
loss and were removed).  No column tiling means no psum strip-sum — no
selection matmul, no bf16 CAST — so the serial epilogue is just a
PSUM->SBUF copy plus squash, with Sqrt as the ONLY ACT function (ACT's
table RAM holds one table; any second function reloads 1.28us
mid-epilogue).

Measured on HW: ~23.7-24.6us end-to-end (run-to-run spread ~1.5-2.5us,
machine-state drift included; interleaved A/B against the column-tiled
strip-sum variant "bp3" shows bp4 ~1us faster), vs 38.0us for the f32
bp2 baseline.  Fixed framework cost (semaphore init/drain storms,
TENSOR_LOAD, ring flush) measures ~14us for a minimal kernel, so the
compute+DMA body is within ~2us of the HBM-rate floor for this
sharding.  Alternate modes kept for reference: "bp3" (column-tiled +
strip-sum, ~24.5-25.5us), "bp2" (f32 8-way batch, 38us), "bp"
(unpacked, 52us), "a2a"/"rs"/"ar"/"ag" (K-sharded + collectives,
87-105us).

The host converts the gathered (256,160) result back to (256, 10, 16).
"""

import numpy as np

B, N, P, L, O = 256, 8, 1152, 10, 16
NCORES = 8
KC = P // 128          # 9 k-chunks of 128 per core
BB = B // NCORES       # 32 batch rows per core in the scatter modes
LO = L * O             # 160

MODE = "bp4"

GP = 4                 # col-tiled k-chunks per PE pass in "bp" mode
NPASS = N * P // 128 // GP   # 18 passes over the full K for one core

# bp3: 4-way batch x 2-way output-capsule sharding, bf16 inputs.
B4 = B // 4            # 64 batch rows per core
O2 = O // 2            # 8 output capsules per core
FO = O2 * L            # 80 output columns per core (f = o_local*10 + l)
KC3 = N * P // 128     # 72 k-chunks of 128
GP3 = 2                # col-tiled k-chunks per PE pass (two 64-col groups)
NP3 = KC3 // GP3       # 36 passes
WXC = GP3 * FO + GP3 * B4   # 288 packed cols per pass: [w | xt]

# bp4: like bp3 but ONE 64-col PE group (no column tiling) — the
# 64-col LDWEIGHTS (~60ns) still hides behind the previous matmul's
# 80-col stream (~67ns) via the PE's weight double-buffer, the psum
# strip-sum disappears, and with it the selection matmul + CAST.
WXC4 = FO + B4         # 144 packed cols per k-chunk: [w | xt]

_cache = {}


def _emit_squash(nc, mybir, post, s, nrows, idx, no=O):
    """Emit squash for an SBUF tile s of shape [nrows, no*L]; returns v tile."""
    f32 = mybir.dt.float32
    nf = no * L
    sq = post.tile([nrows, nf], f32, name=f"sq{idx}")
    m2 = post.tile([nrows, no], f32, name=f"m2{idx}")
    rt = post.tile([nrows, no], f32, name=f"rt{idx}")
    dn = post.tile([nrows, no], f32, name=f"dn{idx}")
    tf = post.tile([nrows, no], f32, name=f"tf{idx}")
    vv = post.tile([nrows, nf], f32, name=f"vv{idx}")
    nc.vector.tensor_mul(sq[:], s[:], s[:])
    nc.vector.reduce_sum(
        m2[:], sq[:].rearrange("b (o l) -> b o l", l=L),
        axis=mybir.AxisListType.X)
    nc.scalar.activation(rt[:], m2[:], mybir.ActivationFunctionType.Sqrt)
    nc.vector.tensor_scalar_add(dn[:], m2[:], 1.0)
    nc.vector.reciprocal(dn[:], dn[:])
    nc.vector.tensor_mul(tf[:], rt[:], dn[:])
    nc.vector.tensor_mul(
        vv[:].rearrange("b (o l) -> b o l", l=L),
        s[:].rearrange("b (o l) -> b o l", l=L),
        tf[:][:, :, None].broadcast_to([nrows, no, L]))
    return vv


def _build(mode=MODE):
    if mode in _cache:
        return _cache[mode]

    import concourse.bacc as bacc
    import concourse.mybir as mybir
    import concourse.tile as tile

    f32 = mybir.dt.float32
    nc = bacc.Bacc("TRN2", target_bir_lowering=False, debug=False,
                   num_devices=NCORES)
    if mode == "bp":
        return _build_bp(nc, mybir)
    if mode == "bp2":
        return _build_bp2(nc, mybir)
    if mode == "bp3":
        return _build_bp3(nc, mybir)
    if mode == "bp4":
        return _build_bp4(nc, mybir)
    if mode == "bp5":
        return _build_bp5(nc, mybir)
    xt_d = nc.dram_tensor("xt", [P, B], f32, kind="ExternalInput").ap()
    w_d = nc.dram_tensor("w", [P, LO], f32, kind="ExternalInput").ap()
    out_rows = BB if mode in ("rs", "a2a") else B
    out_d = nc.dram_tensor("out", [out_rows, LO], f32,
                           kind="ExternalOutput").ap()

    with tile.TileContext(nc) as tc:
        with (
            tc.tile_pool(name="io", bufs=3) as io_pool,
            tc.tile_pool(name="ps", bufs=1, space="PSUM") as ps_pool,
            tc.tile_pool(name="dram", bufs=1, space="DRAM") as dram_pool,
            tc.tile_pool(name="post", bufs=1) as post,
        ):
            xt_v = xt_d.rearrange("(c p) b -> c p b", p=128)
            w_v = w_d.rearrange("(c p) f -> c p f", p=128)
            ps0 = ps_pool.tile([128, LO], f32, name="ps0")
            ps1 = ps_pool.tile([128, LO], f32, name="ps1")
            for c in range(KC):
                xt_t = io_pool.tile([128, B], f32, tag="xt", name=f"xt{c}")
                w_t = io_pool.tile([128, LO], f32, tag="w", name=f"w{c}")
                nc.sync.dma_start(xt_t[:], xt_v[c])
                nc.sync.dma_start(w_t[:], w_v[c])
                nc.tensor.matmul(ps0[:], xt_t[:, 0:128], w_t[:],
                                 start=(c == 0), stop=(c == KC - 1))
                nc.tensor.matmul(ps1[:], xt_t[:, 128:256], w_t[:],
                                 start=(c == 0), stop=(c == KC - 1))

            partial = dram_pool.tile([B, LO], f32, name="partial")
            s0 = post.tile([128, LO], f32, name="s0")
            s1 = post.tile([128, LO], f32, name="s1")
            nc.vector.tensor_copy(s0[:], ps0[:])
            nc.vector.tensor_copy(s1[:], ps1[:])
            nc.sync.dma_start(partial[0:128, :], s0[:])
            nc.sync.dma_start(partial[128:256, :], s1[:])

            rg = [list(range(NCORES))]
            if mode == "ar":
                red = dram_pool.tile([B, LO], f32, name="red",
                                     addr_space="Shared")
                nc.gpsimd.collective_compute(
                    "AllReduce", mybir.AluOpType.add, replica_groups=rg,
                    ins=[partial.opt()], outs=[red.opt()])
                for h in range(2):
                    sh = post.tile([128, LO], f32, name=f"sh{h}")
                    nc.sync.dma_start(sh[:], red[128 * h:128 * (h + 1), :])
                    vv = _emit_squash(nc, mybir, post, sh, 128, h)
                    nc.sync.dma_start(out_d[128 * h:128 * (h + 1), :], vv[:])
            elif mode == "ag":
                red = dram_pool.tile([NCORES * B, LO], f32, name="red",
                                     addr_space="Shared")
                nc.gpsimd.collective_compute(
                    "AllGather", mybir.AluOpType.bypass, replica_groups=rg,
                    ins=[partial.opt()], outs=[red.opt()])
                red_v = red.rearrange("(r b) f -> b r f", b=B)
                for h in range(2):
                    r8 = post.tile([128, NCORES, LO], f32, name=f"r8{h}")
                    nc.sync.dma_start(r8[:], red_v[128 * h:128 * (h + 1)])
                    sh = post.tile([128, LO], f32, name=f"sh{h}")
                    nc.vector.reduce_sum(
                        sh[:], r8[:].rearrange("b r f -> b f r"),
                        axis=mybir.AxisListType.X)
                    vv = _emit_squash(nc, mybir, post, sh, 128, h)
                    nc.sync.dma_start(out_d[128 * h:128 * (h + 1), :], vv[:])
            elif mode == "rs":
                red = dram_pool.tile([BB, LO], f32, name="red")
                nc.gpsimd.collective_compute(
                    "ReduceScatter", mybir.AluOpType.add, replica_groups=rg,
                    ins=[partial.opt()], outs=[red.opt()])
                s = post.tile([BB, LO], f32, name="s")
                nc.sync.dma_start(s[:], red[:])
                vv = _emit_squash(nc, mybir, post, s, BB, 0)
                nc.sync.dma_start(out_d[:], vv[:])
            else:  # a2a
                red = dram_pool.tile([B, LO], f32, name="red")
                nc.gpsimd.collective_compute(
                    "AllToAll", mybir.AluOpType.bypass, replica_groups=rg,
                    ins=[partial.opt()], outs=[red.opt()])
                r8 = post.tile([BB, NCORES, LO], f32, name="r8")
                nc.sync.dma_start(r8[:], red.rearrange("(r b) f -> b r f",
                                                       b=BB))
                s = post.tile([BB, LO], f32, name="s")
                nc.vector.reduce_sum(
                    s[:], r8[:].rearrange("b r f -> b f r"),
                    axis=mybir.AxisListType.X)
                vv = _emit_squash(nc, mybir, post, s, BB, 0)
                nc.sync.dma_start(out_d[:], vv[:])

    nc.compile()
    _cache[mode] = nc
    return nc


def _build_bp(nc, mybir):
    """Batch-parallel: W replicated, batch sharded 8 x 32, no collective.

    PE efficiency at M=32 is recovered with 4x column tiling: each PE pass
    runs 4 k-chunks concurrently in the four 32-column groups of the array,
    accumulating into four disjoint 32-partition strips of one PSUM tile.
    The four strips are partial K-sums, added together on DVE at the end.
    DMA is split across both HWDGE queues (sync + scalar)."""
    import concourse.tile as tile

    f32 = mybir.dt.float32
    K = N * P
    xt_d = nc.dram_tensor("xt", [K, BB], f32, kind="ExternalInput").ap()
    w_d = nc.dram_tensor("w", [K, LO], f32, kind="ExternalInput").ap()
    sel_d = nc.dram_tensor("sel", [128, BB], f32, kind="ExternalInput").ap()
    out_d = nc.dram_tensor("out", [BB, LO], f32, kind="ExternalOutput").ap()

    with tile.TileContext(nc) as tc:
        with (
            tc.tile_pool(name="io", bufs=3) as io_pool,
            tc.tile_pool(name="ps", bufs=1, space="PSUM") as ps_pool,
            tc.tile_pool(name="post", bufs=1) as post,
        ):
            xt_v = xt_d.rearrange("(g j p) m -> g p j m", j=GP, p=128)
            w_v = w_d.rearrange("(g j p) f -> g p j f", j=GP, p=128)
            sel_t = post.tile([128, BB], f32, name="sel_t")
            nc.scalar.dma_start(sel_t[:], sel_d[:])
            ps = ps_pool.tile([128, LO], f32, name="ps")
            for g in range(NPASS):
                xt_t = io_pool.tile([128, GP, BB], f32, tag="xt",
                                    name=f"xt{g}")
                w_t = io_pool.tile([128, GP, LO], f32, tag="w", name=f"w{g}")
                dma_eng = nc.sync if g % 2 == 0 else nc.scalar
                xt_eng = nc.scalar if g % 2 == 0 else nc.sync
                xt_eng.dma_start(xt_t[:], xt_v[g])
                dma_eng.dma_start(w_t[:], w_v[g])
                for j in range(GP):
                    nc.tensor.matmul(
                        ps[32 * j:32 * (j + 1), :], xt_t[:, j, :],
                        w_t[:, j, :], start=(g == 0), stop=(g == NPASS - 1),
                        tile_position=(0, 32 * j))

            # sum the four 32-partition strips: s = sel.T @ sp on the PE
            # (DVE cannot add across base partitions; walrus rejects it).
            sp = post.tile([128, LO], f32, name="sp")
            nc.vector.tensor_copy(sp[:], ps[:])
            ps2 = ps_pool.tile([BB, LO], f32, name="ps2")
            nc.tensor.matmul(ps2[:], sel_t[:], sp[:], start=True, stop=True)
            s = post.tile([BB, LO], f32, name="s")
            nc.vector.tensor_copy(s[:], ps2[:])
            vv = _emit_squash(nc, mybir, post, s, BB, 0)
            nc.sync.dma_start(out_d[:], vv[:])

    nc.compile()
    _cache["bp"] = nc
    return nc


def _build_bp2(nc, mybir):
    """Like bp, but inputs are host-packed so each PE pass's W/xt tile is a
    contiguous DRAM block (per-partition runs of 1280B/512B instead of
    640B/128B), and every W pass-load is split across both HWDGE queues."""
    import concourse.tile as tile

    f32 = mybir.dt.float32
    xt_d = nc.dram_tensor("xt", [128, NPASS * GP * BB], f32,
                          kind="ExternalInput").ap()
    w_d = nc.dram_tensor("w", [NPASS * 128, GP * LO], f32,
                         kind="ExternalInput").ap()
    sel_d = nc.dram_tensor("sel", [128, BB], f32, kind="ExternalInput").ap()
    out_d = nc.dram_tensor("out", [BB, LO], f32, kind="ExternalOutput").ap()

    with tile.TileContext(nc) as tc:
        with (
            tc.tile_pool(name="io", bufs=5) as io_pool,
            tc.tile_pool(name="ps", bufs=1, space="PSUM") as ps_pool,
            tc.tile_pool(name="post", bufs=1) as post,
        ):
            # DMA granularity: PR passes per issue (fewer, larger transfers —
            # each dma_start costs ~670ns of issue time on its HWDGE engine,
            # and the kernel-teardown sem storm scales with instruction count).
            # The first group is a single pass so the PE can start sooner.
            PR = 3
            groups = [1] + [PR] * ((NPASS - 1) // PR) + \
                     ([NPASS - 1 - (NPASS - 1) // PR * PR] or [])
            groups = [n for n in groups if n]
            w_vp = w_d.rearrange("(g p) f -> g p f", p=128)
            sel_t = post.tile([128, BB], f32, name="sel_t")
            nc.scalar.dma_start(sel_t[:], sel_d[:])
            # x is tiny (9.2KB/partition): keep it SBUF-resident, loaded by
            # two early DMAs instead of one per group — fewer issues and no
            # xt dependency in the W streaming pipeline.
            XA = 7 * GP * BB
            xt_all = post.tile([128, NPASS * GP * BB], f32, name="xt_all")
            nc.scalar.dma_start(xt_all[:, 0:XA], xt_d[:, 0:XA])
            ps = ps_pool.tile([128, LO], f32, name="ps")
            # PE warm-up: ~4us of dummy matmuls on the tiny sel tile while
            # the first W loads are in flight, so the HAM un-throttles the
            # PE clock (1.2 -> 2.4 GHz) before the real passes start.
            warm = ps_pool.tile([BB, BB], f32, name="warm")
            for _ in range(10):
                nc.tensor.matmul(warm[:], sel_t[:, 0:BB], sel_t[:, 0:BB],
                                 start=True, stop=True)
            g0 = 0
            for gi, npg in enumerate(groups):
                w_t = io_pool.tile([128, npg, GP * LO], f32, tag="w",
                                   name=f"w{gi}")
                ws = w_vp[g0:g0 + npg].rearrange("h p f -> p h f")
                e0, e1 = (nc.sync, nc.scalar) if gi % 2 == 0 else \
                         (nc.scalar, nc.sync)
                if npg == 1:
                    half = GP * LO // 2
                    e0.dma_start(w_t[:, 0, 0:half], ws[:, 0, 0:half])
                    e1.dma_start(w_t[:, 0, half:], ws[:, 0, half:])
                else:
                    # first-needed pass on e0, rest on e1
                    e0.dma_start(w_t[:, 0:1, :], ws[:, 0:1, :])
                    e1.dma_start(w_t[:, 1:npg, :], ws[:, 1:npg, :])
                if gi == 0:
                    nc.sync.dma_start(xt_all[:, XA:], xt_d[:, XA:])
                for h in range(npg):
                    g = g0 + h
                    for j in range(GP):
                        c = g * GP + j
                        nc.tensor.matmul(
                            ps[32 * j:32 * (j + 1), :],
                            xt_all[:, BB * c:BB * (c + 1)],
                            w_t[:, h, LO * j:LO * (j + 1)],
                            start=(g == 0), stop=(g == NPASS - 1),
                            tile_position=(0, 32 * j))
                g0 += npg

            sp = post.tile([128, LO], f32, name="sp")
            nc.vector.tensor_copy(sp[:], ps[:])
            ps2 = ps_pool.tile([BB, LO], f32, name="ps2")
            nc.tensor.matmul(ps2[:], sel_t[:], sp[:], start=True, stop=True)
            s = post.tile([BB, LO], f32, name="s")
            nc.vector.tensor_copy(s[:], ps2[:])
            vv = _emit_squash(nc, mybir, post, s, BB, 0)
            nc.sync.dma_start(out_d[:], vv[:])

    nc.compile()
    _cache["bp2"] = nc
    return nc


def _build_bp3(nc, mybir):
    """4-way batch x 2-way output-capsule sharding, bf16 inputs.

    Each core computes s[b, f] for 64 batch rows and 80 output columns
    (8 of the 16 o-capsules, all 10 l's; the squash l-reduction stays
    core-local).  Per-core traffic drops from 7.1 MB (bp2) to 2.65 MB:
    bf16 halves the bytes and the 4x2 grid replicates x only 2x and W
    only 4x instead of 8x.

    W and x are host-interleaved into ONE packed stream wx: per PE pass
    g the block [w(2 chunks, 160 cols) | xt(2 chunks, 128 cols)], so DMA
    delivery order == PE consumption order and every transfer is one
    contiguous per-partition run, moved as 12 3-pass dma_starts
    alternating between the two HWDGE paths.  M=64 PE efficiency is
    recovered with 2x column tiling (tile_position=(0,64j)); the two
    64-partition strips are summed by a small selection-matrix matmul
    as in bp2.
    """
    import concourse.tile as tile

    f32 = mybir.dt.float32
    bf16 = mybir.dt.bfloat16
    wx_d = nc.dram_tensor("wx", [128, NP3 * WXC], bf16,
                          kind="ExternalInput").ap()
    sel_d = nc.dram_tensor("sel", [128, B4], bf16, kind="ExternalInput").ap()
    out_d = nc.dram_tensor("out", [B4, FO], f32, kind="ExternalOutput").ap()

    with tile.TileContext(nc) as tc:
        with (
            tc.tile_pool(name="io", bufs=12) as io_pool,
            tc.tile_pool(name="ps", bufs=1, space="PSUM") as ps_pool,
            tc.tile_pool(name="post", bufs=1) as post,
        ):
            # Sequential small groups: one dma_start per group (128
            # descriptors, npg*576B contiguous per-partition runs),
            # alternating between the two HWDGE paths.  The 16 hardware
            # rings serve each source FIFO but arbitrate BETWEEN the
            # two sources in bursts, so LARGE alternated groups arrive
            # out of order and stall the in-order PE consumer; at
            # 3-pass granularity the skew stays below the PE's slack
            # while the alternation fills each source's ring re-arm
            # gaps (measured best of 2/3/4/6-pass x single/dual-source
            # variants, ~320 GB/s/core aggregate — the cap).  No
            # warm-up: the HAM never un-throttles the PE clock
            # mid-kernel (bp2's ramp fired at t=33us, after its
            # matmuls), so warm-up matmuls only delayed the first pass.
            groups = [3] * 12
            assert sum(groups) == NP3
            wx_v = wx_d.rearrange("p (g c) -> p g c", c=WXC)
            sel_t = post.tile([128, B4], bf16, name="sel_t")
            ps = ps_pool.tile([128, FO], f32, name="ps")
            g0 = 0
            for gi, npg in enumerate(groups):
                wx_t = io_pool.tile([128, npg, WXC], bf16, tag="wx",
                                    name=f"wx{gi}")
                e = nc.sync if gi % 2 == 0 else nc.scalar
                e.dma_start(wx_t[:], wx_v[:, g0:g0 + npg])
                if gi == 0:
                    # sel is only needed for the final strip-sum; issue it
                    # on the other HWDGE path so pass-0 data leads.
                    nc.scalar.dma_start(sel_t[:], sel_d[:])
                for h in range(npg):
                    g = g0 + h
                    for j in range(GP3):
                        nc.tensor.matmul(
                            ps[B4 * j:B4 * (j + 1), :],
                            wx_t[:, h, GP3 * FO + B4 * j:
                                 GP3 * FO + B4 * (j + 1)],
                            wx_t[:, h, FO * j:FO * (j + 1)],
                            start=(g == 0), stop=(g == NP3 - 1),
                            tile_position=(0, B4 * j))
                g0 += npg

            # sum the two 64-partition strips: s = sel.T @ sp on the PE
            # (bf16 so the strip-sum stream runs at full rate)
            sp = post.tile([128, FO], bf16, name="sp")
            nc.vector.tensor_copy(sp[:], ps[:])
            ps2 = ps_pool.tile([B4, FO], f32, name="ps2")
            nc.tensor.matmul(ps2[:], sel_t[:], sp[:], start=True, stop=True)
            # ACT's table RAM holds one table: keep Sqrt the only ACT
            # function so its table loads once early, never mid-epilogue.
            s = post.tile([B4, FO], f32, name="s")
            nc.vector.tensor_copy(s[:], ps2[:])
            vv = _emit_squash(nc, mybir, post, s, B4, 0, no=O2)
            # scalar's issue queue is long done by now; sync still owns
            # the end-barrier bookkeeping, so the out store leaves sooner
            # from scalar.
            nc.scalar.dma_start(out_d[:], vv[:])

    nc.compile()
    _cache["bp3"] = nc
    return nc


def _build_bp4(nc, mybir):
    """bp3's sharding/stream with a single 64-col PE group.

    All 72 k-chunk matmuls accumulate into one [64, 80] PSUM tile at
    tile_position (0,0); the 64-col LDWEIGHTS hides behind the previous
    matmul's 80-col stream in the PE weight double-buffer, so the
    cadence matches bp3's column-tiled form while the strip-sum
    (selection matmul + bf16 CAST + sel DMA) disappears from the serial
    epilogue.
    """
    import concourse.tile as tile

    f32 = mybir.dt.float32
    bf16 = mybir.dt.bfloat16
    wx_d = nc.dram_tensor("wx", [128, KC3 * WXC4], bf16,
                          kind="ExternalInput").ap()
    out_d = nc.dram_tensor("out", [B4, FO], f32, kind="ExternalOutput").ap()

    with tile.TileContext(nc) as tc:
        with (
            tc.tile_pool(name="io", bufs=12) as io_pool,
            tc.tile_pool(name="ps", bufs=1, space="PSUM") as ps_pool,
            tc.tile_pool(name="post", bufs=1) as post,
        ):
            # same delivery scheme as bp3: 12 groups (6 chunks each,
            # 1728B contiguous per-partition runs) alternating between
            # the two HWDGE paths.
            groups = [6] * 12
            assert sum(groups) == KC3
            wx_v = wx_d.rearrange("p (g c) -> p g c", c=WXC4)
            ps = ps_pool.tile([B4, FO], f32, name="ps")
            g0 = 0
            for gi, npg in enumerate(groups):
                wx_t = io_pool.tile([128, npg, WXC4], bf16, tag="wx",
                                    name=f"wx{gi}")
                e = nc.sync if gi % 2 == 0 else nc.scalar
                e.dma_start(wx_t[:], wx_v[:, g0:g0 + npg])
                for h in range(npg):
                    c = g0 + h
                    nc.tensor.matmul(
                        ps[:], wx_t[:, h, FO:WXC4], wx_t[:, h, 0:FO],
                        start=(c == 0), stop=(c == KC3 - 1))
                g0 += npg

            s = post.tile([B4, FO], f32, name="s")
            nc.vector.tensor_copy(s[:], ps[:])
            vv = _emit_squash(nc, mybir, post, s, B4, 0, no=O2)
            # single store on scalar: keeping sync out of the output
            # path lets its teardown drain run early, concurrent with
            # the store (a 2-engine split store measured slower).
            nc.scalar.dma_start(out_d[:], vv[:])

    nc.compile()
    _cache["bp4"] = nc
    return nc


def _build_bp5(nc, mybir):
    """bp4 + head-start (KNOWN BROKEN, kept as documentation): group
    0's DMA emitted before TileContext entry, to start the rings ~1us
    before the body scope opens.  Blocked at two layers on this stack:
    the bass_interp sim never fires then_inc completion credits for
    out-of-tile DMAs (deadlock, and bass2jax gates HW on the sim), and
    a drain+sem_inc handshake instead hits a walrus codegen
    INTERNAL_ERROR in generateDynamicDMA.  Do not select without a
    framework change."""
    import concourse.tile as tile

    f32 = mybir.dt.float32
    bf16 = mybir.dt.bfloat16
    wx_d = nc.dram_tensor("wx", [128, KC3 * WXC4], bf16,
                          kind="ExternalInput").ap()
    out_d = nc.dram_tensor("out", [B4, FO], f32, kind="ExternalOutput").ap()
    wx_v = wx_d.rearrange("p (g c) -> p g c", c=WXC4)

    GH = 6
    wx0 = nc.alloc_sbuf_tensor("wx0", [128, GH * WXC4], bf16)
    esem = nc.alloc_semaphore("early_wx0")
    wx0_v = wx0.ap().rearrange("p (h c) -> p h c", c=WXC4)
    nc.sync.dma_start(wx0_v[:], wx_v[:, 0:GH])

    with tile.TileContext(nc) as tc:
        with (
            tc.tile_pool(name="io", bufs=12) as io_pool,
            tc.tile_pool(name="ps", bufs=1, space="PSUM") as ps_pool,
            tc.tile_pool(name="post", bufs=1) as post,
        ):
            groups = [GH] * 11
            assert GH + sum(groups) == KC3
            ps = ps_pool.tile([B4, FO], f32, name="ps")
            # sync quiesces its queues (covers the main-block DMA) and
            # signals; the DMA's own then_inc is not modeled for
            # out-of-tile transfers.
            nc.sync.drain()
            nc.sync.sem_inc(esem, 1)
            nc.tensor.wait_ge(esem, 1)
            for h in range(GH):
                nc.tensor.matmul(
                    ps[:], wx0_v[:, h, FO:WXC4], wx0_v[:, h, 0:FO],
                    start=(h == 0), stop=False)
            g0 = GH
            for gi, npg in enumerate(groups):
                wx_t = io_pool.tile([128, npg, WXC4], bf16, tag="wx",
                                    name=f"wx{gi}")
                e = nc.scalar if gi % 2 == 0 else nc.sync
                e.dma_start(wx_t[:], wx_v[:, g0:g0 + npg])
                for h in range(npg):
                    c = g0 + h
                    nc.tensor.matmul(
                        ps[:], wx_t[:, h, FO:WXC4], wx_t[:, h, 0:FO],
                        start=False, stop=(c == KC3 - 1))
                g0 += npg

            s = post.tile([B4, FO], f32, name="s")
            nc.vector.tensor_copy(s[:], ps[:])
            vv = _emit_squash(nc, mybir, post, s, B4, 0, no=O2)
            nc.scalar.dma_start(out_d[:], vv[:])

    nc.compile()
    _cache["bp5"] = nc
    return nc


def _prep_inputs(x, W, mode=MODE):
    x = np.asarray(x, dtype=np.float32)
    W = np.asarray(W, dtype=np.float32)
    if mode in ("bp4", "bp5"):
        import ml_dtypes
        bf16 = ml_dtypes.bfloat16
        wf = np.ascontiguousarray(
            W[0].transpose(3, 0, 2, 1).reshape(N * P, LO))
        wpass = {}
        for ci in range(2):
            wpass[ci] = wf[:, FO * ci:FO * (ci + 1)].reshape(KC3, 128, FO)
        xpass = {}
        for ri in range(4):
            xt = x[B4 * ri:B4 * (ri + 1)].reshape(B4, N * P).T  # (9216, 64)
            xpass[ri] = xt.reshape(KC3, 128, B4)
        in_maps = []
        for i in range(NCORES):
            ri, ci = i // 2, i % 2
            wx = np.concatenate([wpass[ci], xpass[ri]], axis=2)
            wx = np.ascontiguousarray(
                wx.transpose(1, 0, 2).reshape(128, KC3 * WXC4)).astype(bf16)
            in_maps.append({"wx": wx})
        return in_maps
    if mode == "bp3":
        import ml_dtypes
        bf16 = ml_dtypes.bfloat16
        # wf rows k=(n,p), cols f=o*10+l
        wf = np.ascontiguousarray(
            W[0].transpose(3, 0, 2, 1).reshape(N * P, LO))
        sel = np.zeros((128, B4), np.float32)
        sel[np.arange(128), np.arange(128) % B4] = 1.0
        sel = sel.astype(bf16)
        # per-pass packed blocks, shared pieces computed once
        wpass = {}
        for ci in range(2):
            wc = wf[:, FO * ci:FO * (ci + 1)].reshape(NP3, GP3 * 128, FO)
            wpass[ci] = wc.reshape(NP3, GP3, 128, FO).transpose(
                0, 2, 1, 3).reshape(NP3, 128, GP3 * FO)
        xpass = {}
        for ri in range(4):
            xt = x[B4 * ri:B4 * (ri + 1)].reshape(B4, N * P).T  # (9216, 64)
            xpass[ri] = xt.reshape(NP3, GP3, 128, B4).transpose(
                0, 2, 1, 3).reshape(NP3, 128, GP3 * B4)
        in_maps = []
        for i in range(NCORES):
            ri, ci = i // 2, i % 2
            wx = np.concatenate([wpass[ci], xpass[ri]], axis=2)
            wx = np.ascontiguousarray(
                wx.transpose(1, 0, 2).reshape(128, NP3 * WXC)).astype(bf16)
            in_maps.append({"wx": wx, "sel": sel})
        return in_maps
    if mode == "bp2":
        # pack so each pass's tile is one contiguous DRAM block:
        # packed[g, p, j*D+d] = flat[128*(GP*g+j)+p, d]
        wf = np.ascontiguousarray(
            W[0].transpose(3, 0, 2, 1).reshape(N * P, LO))
        w2 = np.ascontiguousarray(
            wf.reshape(NPASS, GP, 128, LO).transpose(0, 2, 1, 3)
            .reshape(NPASS * 128, GP * LO))
        sel = np.zeros((128, BB), np.float32)
        sel[np.arange(128), np.arange(128) % BB] = 1.0
        in_maps = []
        for i in range(NCORES):
            xt = x[BB * i:BB * (i + 1)].reshape(BB, N * P).T  # (9216, 32)
            x2 = np.ascontiguousarray(
                xt.reshape(NPASS * GP, 128, BB).transpose(1, 0, 2)
                .reshape(128, NPASS * GP * BB))
            in_maps.append({"xt": x2, "w": w2, "sel": sel})
        return in_maps
    if mode == "bp":
        # xt = per-core batch-slice of x, flattened (b, n*p) and transposed;
        # w = full W with rows k=(n,p), cols f=o*10+l — identical per core.
        wf = np.ascontiguousarray(
            W[0].transpose(3, 0, 2, 1).reshape(N * P, LO))    # (9216, 160)
        sel = np.zeros((128, BB), np.float32)
        sel[np.arange(128), np.arange(128) % BB] = 1.0
        in_maps = []
        for i in range(NCORES):
            xs = x[BB * i:BB * (i + 1)].reshape(BB, N * P)
            in_maps.append({"xt": np.ascontiguousarray(xs.T), "w": wf,
                            "sel": sel})
        return in_maps
    in_maps = []
    for i in range(NCORES):
        xt = np.ascontiguousarray(x[:, i, :].T)               # (1152, 256)
        w = np.ascontiguousarray(
            W[0, :, :, :, i].transpose(0, 2, 1).reshape(P, LO))  # (1152, 160)
        in_maps.append({"xt": xt, "w": w})
    return in_maps


def _postprocess(results, mode=MODE):
    if mode in ("bp3", "bp4", "bp5"):
        full = np.zeros((B, LO), np.float32)
        for i in range(NCORES):
            ri, ci = i // 2, i % 2
            full[B4 * ri:B4 * (ri + 1), FO * ci:FO * (ci + 1)] = \
                results[i]["out"]
        return np.ascontiguousarray(
            full.reshape(B, O, L).transpose(0, 2, 1))
    if mode in ("rs", "a2a", "bp", "bp2"):
        full = np.concatenate([results[i]["out"] for i in range(NCORES)],
                              axis=0)
    else:
        full = results[0]["out"]
    return np.ascontiguousarray(
        full.reshape(B, O, L).transpose(0, 2, 1))             # (256, 10, 16)


def kernel(x, W):
    from concourse.bass_utils import run_bass_kernel_spmd

    nc = _build(MODE)
    res = run_bass_kernel_spmd(nc, _prep_inputs(x, W, MODE),
                               core_ids=list(range(NCORES)))
    return _postprocess(res.results)

